# revision 1
# baseline (speedup 1.0000x reference)
"""Trainium2 Bass kernel for nn_BGraphConvolution (BGCN message passing).

Sharding: nodes (rows of x / output) split across 8 NeuronCores (12500 each).
Each adjacency's edges are partitioned by destination row; per 128-dest tile
the edges are col-sorted and split into 4 groups (each with source span
< 32768 so int16 dma_gather indices work). The [N,128] pre_sup / bilinear
difference matrices are AllGathered in bf16 before each SpMM stage.
SpMM = dma_gather of source rows (4 SWDGE queues) + iota-compare one-hot
scatter matrix + bf16 TensorE matmuls accumulating in PSUM.
"""
import numpy as np
import ml_dtypes

N = 100000
D_IN, D_OUT = 256, 128
NCORE = 8
NSH = N // NCORE          # 12500 rows per core
P = 128
NT = (NSH + P - 1) // P   # 98 dest tiles per core (last has 84 rows)
LAST_ROWS = NSH - (NT - 1) * P
NGROUP = 4
MAX_SPAN = 32768
NQ = 4                    # SWDGE queues

bf16 = ml_dtypes.bfloat16


def _build_support_meta(rows_l, cols_l, vals_l):
    """groups[t][g]=(base,span,nvalid); blocks[t][g]=(idx16,rl_f32,val_f32)."""
    order = np.argsort(rows_l, kind="stable")
    r, c, v = rows_l[order], cols_l[order], vals_l[order]
    offs = np.concatenate([[0], np.cumsum(np.bincount(r // P, minlength=NT))])
    groups, blocks = [], []
    for t in range(NT):
        s, e_ = offs[t], offs[t + 1]
        tc, tr, tv = c[s:e_], r[s:e_] - t * P, v[s:e_]
        o2 = np.argsort(tc, kind="stable")
        tc, tr, tv = tc[o2], tr[o2], tv[o2]
        n = len(tc)
        gb = [round(g * n / NGROUP) for g in range(NGROUP + 1)]
        tg, tb = [], []
        for g in range(NGROUP):
            lo, hi = gb[g], gb[g + 1]
            gc, gr, gv = tc[lo:hi], tr[lo:hi], tv[lo:hi]
            nv = len(gc)
            if nv == 0:
                tg.append((0, 1, 0))
                tb.append((np.zeros(0, np.int16), np.zeros(0, np.float32),
                           np.zeros(0, np.float32)))
                continue
            base = int(gc[0])
            span = int(gc[-1]) - base + 1
            tg.append((base, span, nv))
            tb.append(((gc - base).astype(np.int16), gr.astype(np.float32),
                       gv.astype(np.float32)))
        groups.append(tg)
        blocks.append(tb)
    return groups, blocks


def _pack_meta(blocks, cnts):
    """blocks[s][t][g]=(idx,rl,val) -> flat [128, W] arrays (shared layout)."""
    idx_cols, rl_cols, val_cols = [], [], []
    for s in range(7):
        for t in range(NT):
            for g in range(NGROUP):
                cnt = cnts[s][t][g]
                nslot = cnt * P
                gi, gr, gv = blocks[s][t][g]
                nv = len(gi)
                idx = np.zeros(nslot, np.int16)
                idx[:nv] = gi
                rr = np.zeros(nslot, np.float32)
                rr[:nv] = gr
                vv = np.zeros(nslot, np.float32)
                vv[:nv] = gv
                idx_cols.append(np.tile(idx.reshape(cnt * 8, 16).T, (8, 1)))
                rl_cols.append(rr.reshape(cnt, P).T)
                val_cols.append(vv.reshape(cnt, P).T)
    return (np.ascontiguousarray(np.concatenate(idx_cols, axis=1)),
            np.ascontiguousarray(np.concatenate(rl_cols, axis=1)),
            np.ascontiguousarray(np.concatenate(val_cols, axis=1)))


def _build_program(cnts, groups0, idx_w, ch_w):
    import concourse.bass as bass
    import concourse.tile as tile
    from concourse import bacc, mybir, library_config
    from concourse.masks import make_identity
    from contextlib import ExitStack

    fp32 = mybir.dt.float32
    bft = mybir.dt.bfloat16
    KCH = D_IN // P  # 2

    nc = bacc.Bacc("TRN2", target_bir_lowering=False, debug=False,
                   num_devices=NCORE, num_swdge_queues=NQ)
    xt_d = nc.dram_tensor("xt", [D_IN, NSH], fp32, kind="ExternalInput").ap()
    wa_d = nc.dram_tensor("wa", [D_IN, D_OUT], fp32, kind="ExternalInput").ap()
    wb_d = nc.dram_tensor("wb", [D_IN, D_OUT], fp32, kind="ExternalInput").ap()
    w1_d = nc.dram_tensor("w1", [D_OUT, 32], fp32, kind="ExternalInput").ap()
    b1_d = nc.dram_tensor("b1", [1, 32], fp32, kind="ExternalInput").ap()
    w2_d = nc.dram_tensor("w2", [32, 1], fp32, kind="ExternalInput").ap()
    iota_d = nc.dram_tensor("iota", [P, P], bft, kind="ExternalInput").ap()
    idx_d = nc.dram_tensor("idxm", [P, idx_w], mybir.dt.int16, kind="ExternalInput").ap()
    rl_d = nc.dram_tensor("rlm", [P, ch_w], fp32, kind="ExternalInput").ap()
    val_d = nc.dram_tensor("valm", [P, ch_w], fp32, kind="ExternalInput").ap()
    out_d = nc.dram_tensor("out", [NSH, D_OUT], fp32, kind="ExternalOutput").ap()

    qctr = [0]

    def next_q():
        q = qctr[0] % NQ
        qctr[0] += 1
        return q

    max_nch = max(sum(cnts[s][t]) for s in range(7) for t in range(NT))
    sup_off_idx, sup_off_ch = [], []
    io, co = 0, 0
    for s in range(7):
        t_idx, t_ch = [], []
        for t in range(NT):
            t_idx.append(io)
            t_ch.append(co)
            w = sum(cnts[s][t])
            io += w * 8
            co += w
        sup_off_idx.append(t_idx)
        sup_off_ch.append(t_ch)
    assert io == idx_w and co == ch_w, (io, idx_w, co, ch_w)

    with tile.TileContext(nc) as tc, ExitStack() as ctx:
        const_pool = ctx.enter_context(tc.tile_pool(name="const", bufs=1))
        meta_pool = ctx.enter_context(tc.tile_pool(name="meta", bufs=3))
        g_pool = ctx.enter_context(tc.tile_pool(name="g", bufs=3))
        s_pool = ctx.enter_context(tc.tile_pool(name="s", bufs=6))
        o_pool = ctx.enter_context(tc.tile_pool(name="o", bufs=3))
        dram = ctx.enter_context(tc.tile_pool(name="dram", bufs=1, space="DRAM"))

        nc.gpsimd.load_library(library_config.mlp)

        iota_t = const_pool.tile([P, P], bft)
        nc.sync.dma_start(iota_t[:], iota_d[:])
        ident = const_pool.tile([P, P], fp32)
        make_identity(nc, ident[:])
        wa_t = const_pool.tile([P, KCH * D_OUT], fp32, tag="wa")
        wb_t = const_pool.tile([P, KCH * D_OUT], fp32, tag="wb")
        for k in range(KCH):
            nc.sync.dma_start(wa_t[:, k * D_OUT:(k + 1) * D_OUT],
                              wa_d[k * P:(k + 1) * P, :])
            nc.sync.dma_start(wb_t[:, k * D_OUT:(k + 1) * D_OUT],
                              wb_d[k * P:(k + 1) * P, :])
        w1_t = const_pool.tile([P, 32], fp32)
        nc.sync.dma_start(w1_t[:], w1_d[:])
        b1_t = const_pool.tile([1, 32], fp32)
        nc.sync.dma_start(b1_t[:], b1_d[:])
        w2_t = const_pool.tile([32, 1], fp32)
        nc.sync.dma_start(w2_t[:], w2_d[:])
        ones_t = const_pool.tile([1, P], fp32)
        nc.vector.memset(ones_t[:], 1.0)

        p_local = dram.tile([NSH, D_OUT], bft, tag="p_local")
        d1_local = dram.tile([NSH, D_OUT], bft, tag="d1_local")
        d2_local = dram.tile([NSH, D_OUT], bft, tag="d2_local")
        p_full = dram.tile([N, D_OUT], bft, tag="p_full", addr_space="Shared")
        d1_full = dram.tile([N, D_OUT], bft, tag="d1_full", addr_space="Shared")
        d2_full = dram.tile([N, D_OUT], bft, tag="d2_full", addr_space="Shared")

        # ---------- dense phase: pre_sup (transposed orientation) ----------
        with tc.tile_pool(name="dense", bufs=2) as dense_pool, \
             tc.tile_pool(name="dpsum", bufs=1, space="PSUM") as dpsum_pool:
            for t in range(NT):
                rows = P if t < NT - 1 else LAST_ROWS
                sl = slice(t * P, t * P + rows)
                xt_t = dense_pool.tile([P, KCH * P], fp32, tag="xt")
                for k in range(KCH):
                    nc.sync.dma_start(xt_t[:, k * P:k * P + rows],
                                      xt_d[k * P:(k + 1) * P, sl])
                psa = dpsum_pool.tile([P, P], fp32, tag="pa")
                psb = dpsum_pool.tile([P, P], fp32, tag="pb")
                for k in range(KCH):
                    nc.tensor.matmul(psa[:, :rows],
                                     lhsT=wa_t[:, k * D_OUT:(k + 1) * D_OUT],
                                     rhs=xt_t[:, k * P:k * P + rows],
                                     start=(k == 0), stop=(k == KCH - 1))
                    nc.tensor.matmul(psb[:, :rows],
                                     lhsT=wb_t[:, k * D_OUT:(k + 1) * D_OUT],
                                     rhs=xt_t[:, k * P:k * P + rows],
                                     start=(k == 0), stop=(k == KCH - 1))
                a_sb = dense_pool.tile([P, P], fp32, tag="a_sb")
                nc.vector.tensor_copy(a_sb[:, :rows], psa[:, :rows])
                tmp = dense_pool.tile([P, P], fp32, tag="tmp")
                nc.vector.tensor_tensor(out=tmp[:, :rows], in0=a_sb[:, :rows],
                                        in1=psb[:, :rows],
                                        op=mybir.AluOpType.subtract)
                nc.vector.tensor_tensor(out=tmp[:, :rows], in0=tmp[:, :rows],
                                        in1=a_sb[:, :rows],
                                        op=mybir.AluOpType.mult)
                al_sb = dense_pool.tile([P, P], fp32, tag="al_sb")
                nc.vector.tensor_scalar(out=al_sb[:, :rows], in0=tmp[:, :rows],
                                        scalar1=0.5, scalar2=None,
                                        op0=mybir.AluOpType.mult)
                nc.vector.tensor_tensor(out=al_sb[:, :rows],
                                        in0=al_sb[:, :rows],
                                        in1=a_sb[:, :rows],
                                        op=mybir.AluOpType.add)
                z = []
                for zi, comp_sb in enumerate((a_sb, al_sb)):
                    psh = dpsum_pool.tile([32, P], fp32, tag="ph")
                    nc.tensor.matmul(psh[:, :rows], lhsT=w1_t[:],
                                     rhs=comp_sb[:, :rows],
                                     start=True, stop=False)
                    nc.tensor.matmul(psh[:, :rows], lhsT=b1_t[:],
                                     rhs=ones_t[:, :rows],
                                     start=False, stop=True)
                    h_sb = dense_pool.tile([32, P], fp32, tag="h_sb")
                    nc.scalar.activation(h_sb[:, :rows], psh[:, :rows],
                                         mybir.ActivationFunctionType.Tanh)
                    psz = dpsum_pool.tile([1, P], fp32, tag="pz")
                    nc.tensor.matmul(psz[:, :rows], lhsT=w2_t[:],
                                     rhs=h_sb[:, :rows], start=True, stop=True)
                    z_sb = dense_pool.tile([1, P], fp32, tag=f"z{zi}")
                    nc.vector.tensor_copy(z_sb[:, :rows], psz[:, :rows])
                    z.append(z_sb)
                dz = dense_pool.tile([1, P], fp32, tag="dz")
                nc.vector.tensor_tensor(out=dz[:, :rows], in0=z[1][:, :rows],
                                        in1=z[0][:, :rows],
                                        op=mybir.AluOpType.subtract)
                ez = dense_pool.tile([1, P], fp32, tag="ez")
                nc.scalar.activation(ez[:, :rows], dz[:, :rows],
                                     mybir.ActivationFunctionType.Exp)
                nc.vector.tensor_scalar(out=ez[:, :rows], in0=ez[:, :rows],
                                        scalar1=1.0, scalar2=None,
                                        op0=mybir.AluOpType.add)
                atta = dense_pool.tile([1, P], fp32, tag="atta")
                nc.vector.reciprocal(atta[:, :rows], ez[:, :rows])
                attb = dense_pool.tile([P, P], fp32, tag="attb")
                nc.gpsimd.partition_broadcast(attb[:, :rows], atta[:, :rows])
                t1 = dense_pool.tile([P, P], fp32, tag="t1")
                nc.vector.tensor_tensor(out=t1[:, :rows], in0=a_sb[:, :rows],
                                        in1=attb[:, :rows],
                                        op=mybir.AluOpType.mult)
                attb2 = dense_pool.tile([P, P], fp32, tag="attb2")
                nc.vector.tensor_scalar(out=attb2[:, :rows],
                                        in0=attb[:, :rows],
                                        scalar1=-1.0, scalar2=1.0,
                                        op0=mybir.AluOpType.mult,
                                        op1=mybir.AluOpType.add)
                t2 = dense_pool.tile([P, P], fp32, tag="t2")
                nc.vector.tensor_tensor(out=t2[:, :rows], in0=al_sb[:, :rows],
                                        in1=attb2[:, :rows],
                                        op=mybir.AluOpType.mult)
                pst = dense_pool.tile([P, P], fp32, tag="pst")
                nc.vector.tensor_tensor(out=pst[:, :rows], in0=t1[:, :rows],
                                        in1=t2[:, :rows],
                                        op=mybir.AluOpType.add)
                ptp = dpsum_pool.tile([P, P], fp32, tag="ptp")
                nc.tensor.transpose(out=ptp[:rows, :], in_=pst[:, :rows],
                                    identity=ident[:])
                prow = dense_pool.tile([P, P], bft, tag="prow")
                nc.vector.tensor_copy(prow[:rows, :], ptp[:rows, :])
                nc.sync.dma_start(p_local[sl, :], prow[:rows, :])

        rg = [list(range(NCORE))]
        nc.gpsimd.collective_compute(
            "AllGather", mybir.AluOpType.bypass, replica_groups=rg,
            ins=[p_local[:]], outs=[p_full[:]])

        def spmm_tile(s, t, src_full, psum_s, psum_q, first, last):
            nch_t = sum(cnts[s][t])
            ioff = sup_off_idx[s][t]
            choff = sup_off_ch[s][t]
            idxt = meta_pool.tile([P, max_nch * 8], mybir.dt.int16, tag="idxt")
            rlt = meta_pool.tile([P, max_nch], fp32, tag="rlt")
            valt = meta_pool.tile([P, max_nch], fp32, tag="valt")
            nc.sync.dma_start(idxt[:, :nch_t * 8], idx_d[:, ioff:ioff + nch_t * 8])
            nc.sync.dma_start(rlt[:, :nch_t], rl_d[:, choff:choff + nch_t])
            nc.sync.dma_start(valt[:, :nch_t], val_d[:, choff:choff + nch_t])
            gt = g_pool.tile([P, max_nch * P], bft, tag="G")
            g3 = gt[:].rearrange("p (c d) -> p c d", d=P)
            q3 = None
            if psum_q is not None:
                q2 = g_pool.tile([P, max_nch * P], bft, tag="Q")
                q3 = q2[:].rearrange("p (c d) -> p c d", d=P)
            off = 0
            for g in range(NGROUP):
                base, span = groups0[s][t][g]
                cnt = cnts[s][t][g]
                if cnt == 0:
                    continue
                nc.gpsimd.dma_gather(
                    out_ap=g3[:, off:off + cnt, :],
                    in_ap=src_full[base:base + span, :],
                    idxs_ap=idxt[:, off * 8:(off + cnt) * 8],
                    num_idxs=cnt * P, num_idxs_reg=cnt * P, elem_size=D_OUT,
                    single_packet=False, queue_num=next_q(),
                )
                if psum_q is not None:
                    nc.scalar.square(q2[:, off * P:(off + cnt) * P],
                                     gt[:, off * P:(off + cnt) * P])
                off += cnt
            for c in range(nch_t):
                s_t = s_pool.tile([P, P], bft)
                nc.vector.tensor_scalar(
                    out=s_t[:], in0=iota_t[:],
                    scalar1=rlt[:, c:c + 1], scalar2=valt[:, c:c + 1],
                    op0=mybir.AluOpType.is_equal, op1=mybir.AluOpType.mult)
                nc.tensor.matmul(psum_s[:], lhsT=s_t[:], rhs=g3[:, c, :],
                                 start=(first and c == 0),
                                 stop=(last and c == nch_t - 1),
                                 skip_group_check=True)
                if psum_q is not None:
                    nc.tensor.matmul(psum_q[:], lhsT=s_t[:], rhs=q3[:, c, :],
                                     start=(c == 0), stop=(c == nch_t - 1),
                                     skip_group_check=True)

        # ---------- supports 1-4 -> d1, d2 ----------
        with tc.tile_pool(name="psB", bufs=1, space="PSUM") as psB:
            for t in range(NT):
                rows = P if t < NT - 1 else LAST_ROWS
                sl = slice(t * P, t * P + rows)
                sq = {}
                for s in (1, 2, 3, 4):
                    psum_s = psB.tile([P, D_OUT], fp32, tag=f"ps{s}")
                    psum_q = psB.tile([P, D_OUT], fp32, tag=f"pq{s}")
                    spmm_tile(s, t, p_full, psum_s, psum_q, True, True)
                    s_sb = o_pool.tile([P, D_OUT], fp32, tag=f"ssb{s}")
                    q_sb = o_pool.tile([P, D_OUT], fp32, tag=f"qsb{s}")
                    nc.vector.tensor_copy(s_sb[:], psum_s[:])
                    nc.vector.tensor_copy(q_sb[:], psum_q[:])
                    sq[s] = (s_sb, q_sb)
                for dloc, (sa, sb_) in ((d1_local, (1, 3)), (d2_local, (2, 4))):
                    sA, qA = sq[sa]
                    sB, qB = sq[sb_]
                    tA = o_pool.tile([P, D_OUT], fp32, tag="tA")
                    nc.vector.tensor_tensor(out=tA[:], in0=sA[:], in1=sA[:],
                                            op=mybir.AluOpType.mult)
                    nc.vector.tensor_tensor(out=tA[:], in0=tA[:], in1=qA[:],
                                            op=mybir.AluOpType.subtract)
                    tB = o_pool.tile([P, D_OUT], fp32, tag="tB")
                    nc.vector.tensor_tensor(out=tB[:], in0=sB[:], in1=sB[:],
                                            op=mybir.AluOpType.mult)
                    nc.vector.tensor_tensor(out=tB[:], in0=tB[:], in1=qB[:],
                                            op=mybir.AluOpType.subtract)
                    dd = o_pool.tile([P, D_OUT], bft, tag="dd")
                    nc.vector.tensor_tensor(out=dd[:], in0=tA[:], in1=tB[:],
                                            op=mybir.AluOpType.subtract)
                    nc.sync.dma_start(dloc[sl, :], dd[:rows, :])

        nc.gpsimd.collective_compute(
            "AllGather", mybir.AluOpType.bypass, replica_groups=rg,
            ins=[d1_local[:]], outs=[d1_full[:]])
        nc.gpsimd.collective_compute(
            "AllGather", mybir.AluOpType.bypass, replica_groups=rg,
            ins=[d2_local[:]], outs=[d2_full[:]])

        # ---------- final: out = relu(spmm0' + spmm5'(D1) + spmm6'(D2)) ----
        with tc.tile_pool(name="psC", bufs=2, space="PSUM") as psC:
            for t in range(NT):
                rows = P if t < NT - 1 else LAST_ROWS
                sl = slice(t * P, t * P + rows)
                psum_f = psC.tile([P, D_OUT], fp32, tag="pf")
                spmm_tile(0, t, p_full, psum_f, None, True, False)
                spmm_tile(5, t, d1_full, psum_f, None, False, False)
                spmm_tile(6, t, d2_full, psum_f, None, False, True)
                o_sb = o_pool.tile([P, D_OUT], fp32, tag="osb")
                nc.scalar.activation(o_sb[:], psum_f[:],
                                     mybir.ActivationFunctionType.Relu)
                nc.sync.dma_start(out_d[sl, :], o_sb[:rows, :])

    nc.compile()
    return nc


def kernel(x, Wa, Wb, Wc, attn_w1, attn_b1, attn_w2, rows, cols, vals):
    from concourse.bass_utils import run_bass_kernel_spmd

    x = np.asarray(x, np.float32)
    Wa = np.asarray(Wa, np.float32)
    Wb = np.asarray(Wb, np.float32)
    attn_w1 = np.asarray(attn_w1, np.float32)
    attn_b1 = np.asarray(attn_b1, np.float32)
    attn_w2 = np.asarray(attn_w2, np.float32)
    rows = np.asarray(rows)
    cols = np.asarray(cols)
    vals = np.asarray(vals, np.float32)

    # fold output-combination constants into the SpMM values:
    # out = relu(0.5*spmm0(P) + 0.125*spmm5(D1) + 0.125*spmm6(D2)),
    # D1 = (s1^2-q1)-(s3^2-q3) (bilinear 0.5 folded into 5/6 scales).
    vscale = [0.5, 1.0, 1.0, 1.0, 1.0, 0.125, 0.125]

    per_core = []
    for m in range(NCORE):
        lo, hi = m * NSH, (m + 1) * NSH
        sup = []
        for s in range(7):
            mask = (rows[s] >= lo) & (rows[s] < hi)
            rl = (rows[s][mask] - lo).astype(np.int32)
            cl = cols[s][mask].astype(np.int32)
            vl = (vals[s][mask] * vscale[s]).astype(np.float32)
            sup.append(_build_support_meta(rl, cl, vl))
        per_core.append(sup)

    # shared chunk counts and merged (base,span) per (support,tile,group)
    cnts = [[[0] * NGROUP for _ in range(NT)] for _ in range(7)]
    groups0 = [[[None] * NGROUP for _ in range(NT)] for _ in range(7)]
    for s in range(7):
        for t in range(NT):
            for g in range(NGROUP):
                bases, ends, mx = [], [], 1
                for m in range(NCORE):
                    base, span, nv = per_core[m][s][0][t][g]
                    bases.append(base)
                    ends.append(base + span)
                    mx = max(mx, (nv + P - 1) // P)
                b0 = min(bases)
                span0 = min(max(ends) - b0, N - b0)
                assert span0 <= MAX_SPAN, f"merged span {span0}"
                cnts[s][t][g] = mx
                groups0[s][t][g] = (b0, span0)
                for m in range(NCORE):
                    base, _, _ = per_core[m][s][0][t][g]
                    gi, gr, gv = per_core[m][s][1][t][g]
                    per_core[m][s][1][t][g] = (
                        (gi.astype(np.int32) + (base - b0)).astype(np.int16),
                        gr, gv)

    iota_np = np.tile(np.arange(P, dtype=np.float32), (P, 1)).astype(bf16)
    in_maps = []
    idx_w = ch_w = None
    for m in range(NCORE):
        blocks = [per_core[m][s][1] for s in range(7)]
        idx_all, rl_all, val_all = _pack_meta(blocks, cnts)
        idx_w, ch_w = idx_all.shape[1], rl_all.shape[1]
        xt = np.ascontiguousarray(x[m * NSH:(m + 1) * NSH, :].T)
        in_maps.append({
            "xt": xt, "wa": Wa, "wb": Wb, "w1": attn_w1,
            "b1": attn_b1.reshape(1, 32), "w2": attn_w2, "iota": iota_np,
            "idxm": idx_all, "rlm": rl_all, "valm": val_all,
        })

    nc = _build_program(cnts, groups0, idx_w, ch_w)
    res = run_bass_kernel_spmd(nc, in_maps, core_ids=list(range(NCORE)))
    out = np.concatenate([res.results[m]["out"] for m in range(NCORE)], axis=0)
    return np.ascontiguousarray(out.astype(np.float32))

